# revision 2
# baseline (speedup 1.0000x reference)
"""Trainium2 Bass kernel for the FC-SNN (LIF hidden layer + LI readout).

Structure (per core, batch-sharded B=512 -> 64):
  host:   i1[t] is a spike-independent linear filter of x, so it is folded into
          a prefilter of x (exact reparameterization); layout/transpose/pad.
  device: S = (0.1*xfilt) @ w1T  (big matmul, time-parallel, fp32r 1-pass)
          v-scan over t (threshold + reset, the only sequential part)
          oc = z @ w_outT        (spike readout matmul, bf16)
          LI readout (vo/io scans) + max over t
  host:   gather [10,64] per core -> [512,10].

Engine layout (steady state per 4-timestep group):
  PE:   fc1 7x16 matmuls (N=256) + fc_out 16 matmuls   ~13.7us
  DVE:  v-scan decay/reset (2 ops/t on [128,1024] f32)  ~9.4us
  ACT:  PSUM->SBUF copies of S (8 paired copies)        ~5.5us
  Pool: spike matrix (f32->bf16 compare) + LI readout   ~5.5us
  DMA:  x stream on ACT queue, w quads on SP queue      ~2.5us
"""

import sys

if "/opt/trn_rl_repo" not in sys.path:
    sys.path.insert(0, "/opt/trn_rl_repo")

from contextlib import ExitStack

import numpy as np

# problem dims (hardcoded per contract)
T, B, C, Hh, Ww = 64, 512, 1, 28, 28
IN, HID, OUT = 784, 2048, 10
NCORES = 8
BL = B // NCORES            # 64 batch rows per core
TB = T * BL                 # 4096 matmul columns per core
KC = 7                      # contraction chunks: 784 padded to 896 = 7*128
MC = HID // 128             # 16 hidden chunks
NQ = 4                      # w1 DMA quads (4 m-chunks each)
TG = 4                      # time steps per pipeline group
NG = T // TG                # 16 groups
NCOL = TG * BL              # 256 columns per group
NVD = 6                     # vd state ring buffers

# "r1": 1-pass fp32r fc1 (fast, ~6e-3 rel); "bf3": 3-pass bf16 split (near-exact)
MODE = "r1"
TRACE = False

_CACHE = {}
LAST_RESULT = None


def _round_mant(a, mbits):
    """Round fp32 mantissa to mbits (round-to-nearest, matching fp32r pre-round)."""
    ai = np.ascontiguousarray(a, np.float32).view(np.uint32).astype(np.uint64)
    half = np.uint64(1) << np.uint64(22 - mbits)
    mask = np.uint64(0xFFFFFFFF) << np.uint64(23 - mbits)
    return ((ai + half) & mask).astype(np.uint32).view(np.float32)


def _to_bf16(a):
    import ml_dtypes
    return np.ascontiguousarray(a).astype(ml_dtypes.bfloat16)


def _layout_x(arr):
    """[TB, IN] -> [128, KC, TB] (pad IN to 896, p-major k-chunks)."""
    a = np.zeros((KC * 128, TB), arr.dtype)
    a[:IN, :] = arr.T
    return np.ascontiguousarray(a.reshape(KC, 128, TB).transpose(1, 0, 2))


def _layout_w(arr):
    """[IN, HID] -> [128, KC, HID]."""
    a = np.zeros((KC * 128, HID), arr.dtype)
    a[:IN, :] = arr
    return np.ascontiguousarray(a.reshape(KC, 128, HID).transpose(1, 0, 2))


def _layout_w2(arr):
    """[HID, OUT] -> [128, MC*OUT]."""
    return np.ascontiguousarray(
        arr.reshape(MC, 128, OUT).transpose(1, 0, 2).reshape(128, MC * OUT)
    )


def _build_nc(mode, tg=TG, repeat=1, level=5):
    import concourse.bacc as bacc
    import concourse.mybir as mybir
    import concourse.tile as tile

    f32 = mybir.dt.float32
    f32r = mybir.dt.float32r
    bf16 = mybir.dt.bfloat16
    Alu = mybir.AluOpType

    nc = bacc.Bacc("TRN2", debug=False)
    ncol = tg * BL
    ng = T // tg
    nvd = NVD if tg <= 4 else 4
    sgbufs = 2 if tg <= 4 else 1
    MQ = MC // NQ               # m-chunks per w quad

    if mode == "r1":
        xdt = wdt = f32r
        nxs, nws = 1, 1
        passes = [(0, 0)]
    else:
        xdt = wdt = bf16
        nxs, nws = 2, 2
        passes = [(0, 0), (0, 1), (1, 0)]

    xps = [
        nc.declare_dram_parameter(f"x{i}", [128, KC, TB], xdt, isOutput=False)
        for i in range(nxs)
    ]
    wps = [
        nc.declare_dram_parameter(f"w{i}", [128, KC, HID], wdt, isOutput=False)
        for i in range(nws)
    ]
    w2ps = [
        nc.declare_dram_parameter("w20", [128, MC * OUT], bf16, isOutput=False)
    ]
    vmax_p = nc.declare_dram_parameter("vmax", [OUT, BL], f32, isOutput=True)

    with tile.TileContext(nc) as tc, ExitStack() as ctx:
        const = ctx.enter_context(tc.tile_pool(name="const", bufs=1))
        xpool = ctx.enter_context(tc.tile_pool(name="x", bufs=3))
        swpool = ctx.enter_context(tc.tile_pool(name="sw", bufs=2))
        sgpool = ctx.enter_context(tc.tile_pool(name="sg", bufs=sgbufs))
        psS = ctx.enter_context(tc.tile_pool(name="psS", bufs=6, space="PSUM"))
        psO = ctx.enter_context(tc.tile_pool(name="psO", bufs=2, space="PSUM"))

        # persistent tiles; w1 is streamed in NQ quads (4 m-chunks each) split
        # over the two HW DGE queues so group-0 matmuls can start early.
        wts = []
        for i in range(nws):
            quads = [
                const.tile([128, KC * MQ * 128], wdt, tag=f"w{i}q{q}",
                           name=f"w{i}q{q}")
                for q in range(NQ)
            ]
            for q in range(NQ):
                eng = nc.sync if q < NQ // 2 else nc.scalar
                eng.dma_start(
                    quads[q][:].rearrange("p (k h) -> p k h", k=KC),
                    wps[i][:, :, q * MQ * 128:(q + 1) * MQ * 128],
                )
            wts.append(quads)
        w2t = const.tile([128, MC * OUT], bf16, tag="w20", name="w20")
        nc.sync.dma_start(w2t[:], w2ps[0][:, :])
        vd = [const.tile([128, MC * BL], f32, tag=f"vd{i}", name=f"vd{i}") for i in range(nvd)]
        vr = const.tile([128, MC * BL], f32, tag="vr", name="vr")
        js = const.tile([OUT, BL], f32, tag="js", name="js")
        vo = const.tile([OUT, BL], f32, tag="vo", name="vo")
        vmax_t = const.tile([OUT, BL], f32, tag="vmax", name="vmax")

        def dma_x(g):
            tiles = []
            for si in range(nxs):
                xt = xpool.tile([128, KC * ncol], xdt, tag=f"x{si}", name=f"x{si}")
                nc.scalar.dma_start(
                    xt[:].rearrange("p (k c) -> p k c", k=KC),
                    xps[si][:, :, g * ncol:(g + 1) * ncol],
                )
                tiles.append(xt)
            return tiles

        def fc1(g, xts):
            sw = swpool.tile([128, tg * MC * BL], f32, tag="swin", name="swin")
            for mp in range(MC // 2):
                ps = psS.tile([128, 2 * ncol], f32, tag="psS", name="psS")
                for j in range(2):
                    m = 2 * mp + j
                    idx, last = 0, len(passes) * KC - 1
                    for (xi, wi) in passes:
                        wq = wts[wi][m // MQ][:].rearrange(
                            "p (k h) -> p k h", k=KC
                        )
                        xap = xts[xi][:].rearrange("p (k c) -> p k c", k=KC)
                        for k in range(KC):
                            lhsT = wq[:, k, (m % MQ) * 128:(m % MQ + 1) * 128]
                            rhs = xap[:, k, :]
                            nc.tensor.matmul(
                                ps[:, j * ncol:(j + 1) * ncol], lhsT, rhs,
                                start=(idx == 0), stop=(idx == last),
                            )
                            idx += 1
                dst = sw[:].rearrange(
                    "p (t m b) -> p t m b", t=tg, m=MC
                )[:, :, 2 * mp:2 * mp + 2, :]
                src = ps[:].rearrange("p (m t b) -> p t m b", m=2, t=tg)
                nc.scalar.copy(dst, src)
            return sw

        def scan(g, sw):
            sg = sgpool.tile([128, tg * MC * BL], bf16, tag="sgn", name="sgn")
            w = MC * BL
            if level < 2:
                return sg
            for tloc in range(tg):
                t = g * tg + tloc
                a = vd[t % nvd]
                b = vd[(t + 1) % nvd]
                nc.vector.scalar_tensor_tensor(
                    vr[:], a[:], 0.5, a[:], op0=Alu.is_le, op1=Alu.mult
                )
                if level >= 3:
                    nc.gpsimd.tensor_scalar(
                        sg[:, tloc * w:(tloc + 1) * w], a[:], 0.5, None,
                        op0=Alu.is_gt
                    )
                nc.vector.scalar_tensor_tensor(
                    b[:], vr[:], 0.9, sw[:, tloc * w:(tloc + 1) * w],
                    op0=Alu.mult, op1=Alu.add,
                )
            return sg

        def fc_out(g, sg):
            po = psO.tile([OUT, ncol], f32, tag="psO", name="psO")
            if level < 4:
                return po
            sgap = sg[:].rearrange("p (t m b) -> p t m b", t=tg, m=MC)
            for m in range(MC):
                rhs = sgap[:, :, m, :]
                lhsT = w2t[:, m * OUT:(m + 1) * OUT]
                nc.tensor.matmul(
                    po[:], lhsT, rhs, start=(m == 0), stop=(m == MC - 1)
                )
            return po

        def readout(g, po):
            if level < 5:
                return
            for tloc in range(tg):
                nc.gpsimd.scalar_tensor_tensor(
                    vo[:], vo[:], 0.9, js[:], op0=Alu.mult, op1=Alu.add
                )
                nc.gpsimd.tensor_tensor(vmax_t[:], vmax_t[:], vo[:], op=Alu.max)
                nc.gpsimd.scalar_tensor_tensor(
                    js[:], js[:], 0.8, po[:, tloc * BL:(tloc + 1) * BL],
                    op0=Alu.mult, op1=Alu.add,
                )

        def whole_body():
            # init state, then software-pipelined emission: fc1 leads the
            # scan by 1 group, the fc_out/readout chain trails by 2 so no
            # engine head-of-line blocks another.
            nc.vector.memset(vd[0][:], 0.0)
            nc.gpsimd.memset(js[:], 0.0)
            nc.gpsimd.memset(vo[:], 0.0)
            nc.gpsimd.memset(vmax_t[:], 0.0)
            xts = dma_x(0)
            sws, sgs, pos = {}, {}, {}
            sws[0] = fc1(0, xts)
            for g in range(1, ng):
                xts = dma_x(g)
                sws[g] = fc1(g, xts)
                sgs[g - 1] = scan(g - 1, sws.pop(g - 1))
                if g >= 2:
                    pos[g - 2] = fc_out(g - 2, sgs.pop(g - 2))
                    readout(g - 2, pos.pop(g - 2))
            sgs[ng - 1] = scan(ng - 1, sws.pop(ng - 1))
            for g in (ng - 2, ng - 1):
                pos[g] = fc_out(g, sgs.pop(g))
                readout(g, pos.pop(g))
            nc.sync.dma_start(vmax_p[:, :], vmax_t[:])

        if repeat > 1:
            with tc.For_i(0, repeat, 1):
                whole_body()
        else:
            whole_body()

    nc.compile()
    return nc


def _prep_inputs(x, w1, w_out, mode):
    x = np.ascontiguousarray(x, np.float32).reshape(T, B, IN)
    # i1[t] = 0.8*i1[t-1] + x[t] @ w1T  ==  prefilter(x)[t] @ w1T
    xf = np.empty_like(x)
    acc = np.zeros((B, IN), np.float32)
    e8 = np.float32(0.8)
    for t in range(T):
        acc = e8 * acc + x[t]
        xf[t] = acc
    xs = np.float32(0.1) * xf                       # S = xs @ w1T
    w1T = np.ascontiguousarray(w1, np.float32).T    # [IN, HID]

    if mode == "r1":
        xparts = [_round_mant(xs, 11)]
        wparts = [_layout_w(_round_mant(w1T, 11))]
    else:
        xh = _to_bf16(xs)
        xl = _to_bf16(xs - xh.astype(np.float32))
        xparts = [xh, xl]
        wh = _to_bf16(w1T)
        wl = _to_bf16(w1T - wh.astype(np.float32))
        wparts = [_layout_w(wh), _layout_w(wl)]

    w2 = np.float32(0.1) * np.ascontiguousarray(w_out, np.float32).T  # [HID,OUT]

    common = {
        "w20": _layout_w2(_to_bf16(w2)),
    }
    for i, wp in enumerate(wparts):
        common[f"w{i}"] = wp

    in_maps = []
    for c in range(NCORES):
        m = dict(common)
        for i, xp in enumerate(xparts):
            xc = xp[:, c * BL:(c + 1) * BL, :].reshape(TB, IN)
            m[f"x{i}"] = _layout_x(xc)
        in_maps.append(m)
    return in_maps


def kernel(x, w1, w_out):
    global LAST_RESULT
    from concourse.bass_utils import run_bass_kernel_spmd

    if MODE not in _CACHE:
        _CACHE[MODE] = _build_nc(MODE, tg=TG)
    nc = _CACHE[MODE]
    in_maps = _prep_inputs(np.asarray(x), np.asarray(w1), np.asarray(w_out), MODE)
    res = run_bass_kernel_spmd(nc, in_maps, list(range(NCORES)), trace=TRACE)
    LAST_RESULT = res
    out = np.empty((B, OUT), np.float32)
    for c in range(NCORES):
        out[c * BL:(c + 1) * BL, :] = np.asarray(res.results[c]["vmax"]).T
    return out


# revision 14
# speedup vs baseline: 4.8118x; 4.8118x over previous
"""Trainium2 Bass kernel for the FC-SNN (LIF hidden layer + LI readout).

Structure (per core, batch-sharded B=512 -> 64):
  host:   i1[t] is a spike-independent linear filter of x, so it is folded into
          a prefilter of x (exact reparameterization); layout/transpose/pad.
  device: S = (0.1*xfilt) @ w1T  (big matmul, time-parallel, fp32r 1-pass)
          v-scan over t (threshold + reset, the only sequential part)
          oc = z @ w_outT        (spike readout matmul, bf16)
          LI readout (vo/io scans) + max over t
  host:   gather [10,64] per core -> [512,10].

Engine layout (steady state per 4-timestep group):
  PE:   fc1 7x16 matmuls (N=256) + fc_out 16 matmuls   ~13.7us
  DVE:  v-scan decay/reset (2 ops/t on [128,1024] f32)  ~9.4us
  ACT:  PSUM->SBUF copies of S (8 paired copies)        ~5.5us
  Pool: spike matrix (f32->bf16 compare) + LI readout   ~5.5us
  DMA:  x stream on ACT queue, w quads on SP queue      ~2.5us
"""

import sys

if "/opt/trn_rl_repo" not in sys.path:
    sys.path.insert(0, "/opt/trn_rl_repo")

from contextlib import ExitStack

import numpy as np

# problem dims (hardcoded per contract)
T, B, C, Hh, Ww = 64, 512, 1, 28, 28
IN, HID, OUT = 784, 2048, 10
NCORES = 8
BL = B // NCORES            # 64 batch rows per core
TB = T * BL                 # 4096 matmul columns per core
KC = 7                      # contraction chunks: 784 padded to 896 = 7*128
MC = HID // 128             # 16 hidden chunks
NQ = 4                      # w1 DMA quads (4 m-chunks each)
TG = 4                      # time steps per pipeline group
NG = T // TG                # 16 groups
NCOL = TG * BL              # 256 columns per group
NVD = 6                     # vd state ring buffers

# "r1": 1-pass fp32r fc1 (fast, ~6e-3 rel); "bf3": 3-pass bf16 split (near-exact)
MODE = "r1"
TRACE = False

_CACHE = {}
LAST_RESULT = None


def _round_mant(a, mbits):
    """Round fp32 mantissa to mbits (round-to-nearest, matching fp32r pre-round)."""
    ai = np.ascontiguousarray(a, np.float32).view(np.uint32).astype(np.uint64)
    half = np.uint64(1) << np.uint64(22 - mbits)
    mask = np.uint64(0xFFFFFFFF) << np.uint64(23 - mbits)
    return ((ai + half) & mask).astype(np.uint32).view(np.float32)


def _to_bf16(a):
    import ml_dtypes
    return np.ascontiguousarray(a).astype(ml_dtypes.bfloat16)


def _layout_x(arr):
    """[TB, IN] -> [128, KC, TB] (pad IN to 896, p-major k-chunks)."""
    a = np.zeros((KC * 128, TB), arr.dtype)
    a[:IN, :] = arr.T
    return np.ascontiguousarray(a.reshape(KC, 128, TB).transpose(1, 0, 2))


def _layout_w(arr):
    """[IN, HID] -> [128, KC, HID]."""
    a = np.zeros((KC * 128, HID), arr.dtype)
    a[:IN, :] = arr
    return np.ascontiguousarray(a.reshape(KC, 128, HID).transpose(1, 0, 2))


def _layout_w2(arr):
    """[HID, OUT] -> [128, MC*OUT]."""
    return np.ascontiguousarray(
        arr.reshape(MC, 128, OUT).transpose(1, 0, 2).reshape(128, MC * OUT)
    )


def _build_nc(mode, tg=TG, repeat=1, level=5):
    import concourse.bacc as bacc
    import concourse.mybir as mybir
    import concourse.tile as tile

    f32 = mybir.dt.float32
    f32r = mybir.dt.float32r
    bf16 = mybir.dt.bfloat16
    f16 = mybir.dt.float16
    Alu = mybir.AluOpType

    nc = bacc.Bacc("TRN2", debug=False)
    ncol = tg * BL
    ng = T // tg
    nvd = NVD if tg <= 4 else 4
    sgbufs = 2 if tg <= 4 else 1
    MQ = MC // NQ               # m-chunks per w quad

    if mode == "r1":
        xdt = wdt = f32r
        nxs, nws = 1, 1
        passes = [(0, 0)]
    else:
        xdt = wdt = bf16
        nxs, nws = 2, 2
        passes = [(0, 0), (0, 1), (1, 0)]

    xps = [
        nc.declare_dram_parameter(f"x{i}", [128, KC, TB], xdt, isOutput=False)
        for i in range(nxs)
    ]
    wps = [
        nc.declare_dram_parameter(f"w{i}", [128, KC, HID], wdt, isOutput=False)
        for i in range(nws)
    ]
    w2ps = [
        nc.declare_dram_parameter("w20", [128, MC * OUT], f16, isOutput=False)
    ]
    vmax_p = nc.declare_dram_parameter("vmax", [OUT, BL], f32, isOutput=True)

    with tile.TileContext(nc) as tc, ExitStack() as ctx:
        const = ctx.enter_context(tc.tile_pool(name="const", bufs=1))
        xpool = ctx.enter_context(tc.tile_pool(name="x", bufs=3))
        swpool = ctx.enter_context(tc.tile_pool(name="sw", bufs=2))
        sgpool = ctx.enter_context(tc.tile_pool(name="sg", bufs=sgbufs))
        ocpool = ctx.enter_context(tc.tile_pool(name="oc", bufs=2))
        psS = ctx.enter_context(tc.tile_pool(name="psS", bufs=6, space="PSUM"))
        psO = ctx.enter_context(tc.tile_pool(name="psO", bufs=2, space="PSUM"))

        # persistent tiles; w1 is streamed in NQ quads (4 m-chunks each) split
        # over the two HW DGE queues so group-0 matmuls can start early.
        wts = []
        for i in range(nws):
            quads = [
                const.tile([128, KC * MQ * 128], wdt, tag=f"w{i}q{q}",
                           name=f"w{i}q{q}")
                for q in range(NQ)
            ]
            for q in range(NQ):
                eng = nc.sync if q < NQ // 2 else nc.scalar
                eng.dma_start(
                    quads[q][:].rearrange("p (k h) -> p k h", k=KC),
                    wps[i][:, :, q * MQ * 128:(q + 1) * MQ * 128],
                )
            wts.append(quads)
        w2t = const.tile([128, MC * OUT], f16, tag="w20", name="w20")
        nc.sync.dma_start(w2t[:], w2ps[0][:, :])
        vd = [const.tile([128, MC * BL], f32, tag=f"vd{i}", name=f"vd{i}") for i in range(nvd)]
        vr = const.tile([128, MC * BL], f32, tag="vr", name="vr")
        js = const.tile([OUT, BL], f32, tag="js", name="js")
        vo = const.tile([OUT, BL], f32, tag="vo", name="vo")
        vmax_t = const.tile([OUT, BL], f32, tag="vmax", name="vmax")

        def dma_x(g):
            tiles = []
            for si in range(nxs):
                xt = xpool.tile([128, KC * ncol], xdt, tag=f"x{si}", name=f"x{si}")
                nc.scalar.dma_start(
                    xt[:].rearrange("p (k c) -> p k c", k=KC),
                    xps[si][:, :, g * ncol:(g + 1) * ncol],
                )
                tiles.append(xt)
            return tiles

        def fc1(g, xts):
            sw = swpool.tile([128, tg * MC * BL], f32, tag="swin", name="swin")
            for mp in range(MC // 2):
                ps = psS.tile([128, 2 * ncol], f32, tag="psS", name="psS")
                for j in range(2):
                    m = 2 * mp + j
                    idx, last = 0, len(passes) * KC - 1
                    for (xi, wi) in passes:
                        wq = wts[wi][m // MQ][:].rearrange(
                            "p (k h) -> p k h", k=KC
                        )
                        xap = xts[xi][:].rearrange("p (k c) -> p k c", k=KC)
                        for k in range(KC):
                            lhsT = wq[:, k, (m % MQ) * 128:(m % MQ + 1) * 128]
                            rhs = xap[:, k, :]
                            nc.tensor.matmul(
                                ps[:, j * ncol:(j + 1) * ncol], lhsT, rhs,
                                start=(idx == 0), stop=(idx == last),
                            )
                            idx += 1
                dst = sw[:].rearrange(
                    "p (t m b) -> p t m b", t=tg, m=MC
                )[:, :, 2 * mp:2 * mp + 2, :]
                src = ps[:].rearrange("p (m t b) -> p t m b", m=2, t=tg)
                nc.scalar.copy(dst, src)
            return sw

        def scan(g, sw):
            # LIF update, f32 state (fp16 state costs ~1e-2 extra error and
            # buys nothing: PE is the binding engine, not DVE):
            #   vr = (a <= 0.5) * a; b = 0.9*vr + sw; sg = (a > 0.5) [fp16]
            sg = sgpool.tile([128, tg * MC * BL], f16, tag="sgn", name="sgn")
            w = MC * BL
            if level < 2:
                return sg
            for tloc in range(tg):
                t = g * tg + tloc
                a = vd[t % nvd]
                b = vd[(t + 1) % nvd]
                nc.vector.scalar_tensor_tensor(
                    vr[:], a[:], 0.5, a[:], op0=Alu.is_le, op1=Alu.mult
                )
                if level >= 3:
                    nc.vector.tensor_scalar(
                        sg[:, tloc * w:(tloc + 1) * w], a[:], 0.5, None,
                        op0=Alu.is_gt
                    )
                nc.vector.scalar_tensor_tensor(
                    b[:], vr[:], 0.9, sw[:, tloc * w:(tloc + 1) * w],
                    op0=Alu.mult, op1=Alu.add,
                )
            return sg

        def fc_out(g, sg):
            oc = ocpool.tile([OUT, ncol], f32, tag="ocs", name="ocs")
            if level < 4:
                return oc
            po = psO.tile([OUT, ncol], f32, tag="psO", name="psO")
            sgap = sg[:].rearrange("p (t m b) -> p t m b", t=tg, m=MC)
            for m in range(MC):
                rhs = sgap[:, :, m, :]
                lhsT = w2t[:, m * OUT:(m + 1) * OUT]
                nc.tensor.matmul(
                    po[:], lhsT, rhs, start=(m == 0), stop=(m == MC - 1)
                )
            # Pool (gpsimd) cannot read PSUM; stage oc through SBUF on ACT
            nc.scalar.copy(oc[:], po[:])
            return oc

        def readout(g, oc):
            if level < 5:
                return
            for tloc in range(tg):
                nc.vector.scalar_tensor_tensor(
                    vo[:], vo[:], 0.9, js[:], op0=Alu.mult, op1=Alu.add
                )
                nc.vector.tensor_tensor(vmax_t[:], vmax_t[:], vo[:], op=Alu.max)
                nc.vector.scalar_tensor_tensor(
                    js[:], js[:], 0.8, oc[:, tloc * BL:(tloc + 1) * BL],
                    op0=Alu.mult, op1=Alu.add,
                )

        def whole_body():
            # init state, then software-pipelined emission: fc1 leads the
            # scan by 1 group, the fc_out/readout chain trails by 2 so no
            # engine head-of-line blocks another.
            nc.vector.memset(vd[0][:], 0.0)
            nc.vector.memset(js[:], 0.0)
            nc.vector.memset(vo[:], 0.0)
            nc.vector.memset(vmax_t[:], 0.0)
            xts = dma_x(0)
            sws, sgs, pos = {}, {}, {}
            sws[0] = fc1(0, xts)
            for g in range(1, ng):
                xts = dma_x(g)
                sws[g] = fc1(g, xts)
                sgs[g - 1] = scan(g - 1, sws.pop(g - 1))
                if g >= 2:
                    pos[g - 2] = fc_out(g - 2, sgs.pop(g - 2))
                    readout(g - 2, pos.pop(g - 2))
            sgs[ng - 1] = scan(ng - 1, sws.pop(ng - 1))
            for g in (ng - 2, ng - 1):
                pos[g] = fc_out(g, sgs.pop(g))
                readout(g, pos.pop(g))
            nc.sync.dma_start(vmax_p[:, :], vmax_t[:])

        if repeat > 1:
            with tc.For_i(0, repeat, 1):
                whole_body()
        else:
            whole_body()

    nc.compile()
    return nc


def _prep_inputs(x, w1, w_out, mode):
    x = np.ascontiguousarray(x, np.float32).reshape(T, B, IN)
    # i1[t] = 0.8*i1[t-1] + x[t] @ w1T  ==  prefilter(x)[t] @ w1T
    xf = np.empty_like(x)
    acc = np.zeros((B, IN), np.float32)
    e8 = np.float32(0.8)
    for t in range(T):
        acc = e8 * acc + x[t]
        xf[t] = acc
    xs = np.float32(0.1) * xf                       # S = xs @ w1T
    w1T = np.ascontiguousarray(w1, np.float32).T    # [IN, HID]

    if mode == "r1":
        xparts = [_round_mant(xs, 11)]
        wparts = [_layout_w(_round_mant(w1T, 11))]
    else:
        xh = _to_bf16(xs)
        xl = _to_bf16(xs - xh.astype(np.float32))
        xparts = [xh, xl]
        wh = _to_bf16(w1T)
        wl = _to_bf16(w1T - wh.astype(np.float32))
        wparts = [_layout_w(wh), _layout_w(wl)]

    w2 = np.float32(0.1) * np.ascontiguousarray(w_out, np.float32).T  # [HID,OUT]

    common = {
        "w20": _layout_w2(w2.astype(np.float16)),
    }
    for i, wp in enumerate(wparts):
        common[f"w{i}"] = wp

    in_maps = []
    for c in range(NCORES):
        m = dict(common)
        for i, xp in enumerate(xparts):
            xc = xp[:, c * BL:(c + 1) * BL, :].reshape(TB, IN)
            m[f"x{i}"] = _layout_x(xc)
        in_maps.append(m)
    return in_maps


def kernel(x, w1, w_out):
    global LAST_RESULT
    from concourse.bass_utils import run_bass_kernel_spmd

    if MODE not in _CACHE:
        _CACHE[MODE] = _build_nc(MODE, tg=TG)
    nc = _CACHE[MODE]
    in_maps = _prep_inputs(np.asarray(x), np.asarray(w1), np.asarray(w_out), MODE)
    res = run_bass_kernel_spmd(nc, in_maps, list(range(NCORES)), trace=TRACE)
    LAST_RESULT = res
    out = np.empty((B, OUT), np.float32)
    for c in range(NCORES):
        out[c * BL:(c + 1) * BL, :] = np.asarray(res.results[c]["vmax"]).T
    return out
